# revision 1
# baseline (speedup 1.0000x reference)
"""FCOS loss kernel for 8 TRN2 NeuronCores (self-contained).

Sharding: data-parallel over batch B=16 -> 8 cores x 2 images.

Device algorithm (per core, per image):
  The FCOS min-area box assignment is computed with TensorEngine matmuls over
  separable interval-indicator tables instead of an O(N*M) scan:
    valid[y,x,m] = v1[m,y]*ua[m,x] + va[m,y]*ub[m,x]      (non-negative terms)
  where u1/v1 are the "in-box & below-high-limit" interval indicators,
  u2/v2 the "below-low-limit" ones, ua=u1&~u2, ub=u1&u2, va=v1&~v2.
  Each box m carries a priority weight w_m = 2^(LB*slot_m + E0), slots
  assigned on host so that any two boxes whose valid regions can overlap get
  distinct slots, ordered so smaller area => higher slot.  Then
    D[pix]   = sum_m valid*w_m            (5 matmul channels: D, w*x0, w*y0,
    Nc[pix]  = sum_m valid*w_m*coord_m     w*x2, w*y2)
  and matched coord = Nc/D to ~2^-LB relative contamination; pos = D>0 exact.
  Decode + iou/giou/varifocal + reductions are elementwise passes in a packed
  [128, 320] per-pixel layout:
    L0a: rows 0:128  cols 0:160    (level0 y=0..127, x=col)
    L0b: rows 0:32   cols 160:320  (level0 y=128+row, x=col-160)
    L1 : rows 32:112 cols 160:240  (level1 y=row-32, x=col-160)
    L2 : rows 32:72  cols 240:280  (level2 y=row-32, x=col-240)
  Garbage cells are zeroed/defused so they contribute exactly 0 to all sums.
Output per core: [cls_sum, reg_sum, npos]; final normalization on host.
"""
import sys
import numpy as np

for _p in ("/opt/trn_rl_repo", "/root/.axon_site/_ro/trn_rl_repo"):
    if _p not in sys.path:
        sys.path.insert(0, _p)

STRIDES = (4, 8, 16)
LIMITS = np.array([[-1.0, 64.0], [64.0, 128.0], [128.0, 99999.0]], dtype=np.float32)
SIZES = ((160, 160), (80, 80), (40, 40))
B, M = 16, 64
NCORES = 8
IMGS = 2                      # images per core
E0 = -110                     # base exponent of priority weights
PTC = 320                     # per-pixel tile cols
TBC = 280                     # table cols: 160+80+40

# ---------------------------------------------------------------- host prep


def _assign_slots(boxes64):
    """Greedy slot assignment on the conflict graph (host control-plane).
    boxes64: [M,4] f32. Returns slots [M] int."""
    bx = boxes64.astype(np.float64)
    areas = (bx[:, 2] - bx[:, 0]) * (bx[:, 3] - bx[:, 1])
    conflict = np.zeros((M, M), dtype=bool)
    for low, high in LIMITS:
        ax = np.maximum(bx[:, 0], bx[:, 2] - high)
        bxx = np.minimum(bx[:, 2], bx[:, 0] + high)
        ay = np.maximum(bx[:, 1], bx[:, 3] - high)
        by = np.minimum(bx[:, 3], bx[:, 1] + high)
        ne = (ax < bxx) & (ay < by)
        ox = (ax[:, None] < bxx[None, :]) & (ax[None, :] < bxx[:, None])
        oy = (ay[:, None] < by[None, :]) & (ay[None, :] < by[:, None])
        conflict |= ox & oy & ne[:, None] & ne[None, :]
    order = sorted(range(M), key=lambda m: (-areas[m], -m))
    slots = np.zeros(M, dtype=np.int64)
    done = []
    for m in order:
        cs = [slots[k] for k in done if conflict[m, k]]
        slots[m] = (max(cs) + 1) if cs else 0
        done.append(m)
    return slots


def _slot_weights(boxes64):
    """Priority weights per box for one image."""
    slots = _assign_slots(boxes64)
    smax = int(slots.max())
    lb = min(24, max(4, (120 - E0 - 14) // max(1, smax)))   # keep max exp < ~2^110
    return np.exp2(lb * slots + E0).astype(np.float32), lb


def _grid(n, stride):
    return ((np.arange(n, dtype=np.float32) + np.float32(0.5))
            * np.float32(stride)).astype(np.float32)


def _pt_pack(planes):
    """planes: list of 3 arrays [H,W] (levels) -> packed [128, 320] f32."""
    out = np.zeros((128, PTC), dtype=np.float32)
    p0, p1, p2 = planes
    out[0:128, 0:160] = p0[0:128, :]
    out[0:32, 160:320] = p0[128:160, :]
    out[32:112, 160:240] = p1
    out[32:72, 240:280] = p2
    return out


def _const_tensors():
    """Input-independent constants."""
    gx = [_grid(W, s) for s, (H, W) in zip(STRIDES, SIZES)]
    gy = [_grid(H, s) for s, (H, W) in zip(STRIDES, SIZES)]
    gridx = np.concatenate(gx)          # [280]
    gridy = np.concatenate(gy)
    GRIDX = np.repeat(gridx[None, :], 128, 0).astype(np.float32)
    GRIDY = np.repeat(gridy[None, :], 128, 0).astype(np.float32)
    HC = np.concatenate([np.full(W, LIMITS[l, 1], np.float32)
                         for l, (H, W) in enumerate(SIZES)])
    LC = np.concatenate([np.full(W, LIMITS[l, 0], np.float32)
                         for l, (H, W) in enumerate(SIZES)])
    HCT = np.repeat(HC[None, :], 128, 0).astype(np.float32)
    LCT = np.repeat(LC[None, :], 128, 0).astype(np.float32)
    XS = np.tile(_pt_pack([np.repeat(g[None, :], H, 0)
                           for g, (H, W) in zip(gx, SIZES)]), (1, 2))
    YS = np.tile(_pt_pack([np.repeat(g[:, None], W, 1)
                           for g, (H, W) in zip(gy, SIZES)]), (1, 2))
    ONES = np.ones((128, 1), dtype=np.float32)
    return GRIDX, GRIDY, HCT, LCT, XS, YS, ONES


# ------------------------------------------------------------- bass builder

_CACHE = {}


def _build_nc(rep=1):
    import concourse.bacc as bacc
    import concourse.mybir as mybir
    from concourse.tile import TileContext

    dt = mybir.dt
    f32 = dt.float32
    bf16 = dt.bfloat16
    Alu = mybir.AluOpType
    Act = mybir.ActivationFunctionType

    nc = bacc.Bacc("TRN2", num_devices=NCORES)

    # ---- dram parameters (per-core shards + constants)
    d_cls = [nc.dram_tensor(f"cls{l}", [IMGS, 1, H, W], f32, kind="ExternalInput")
             for l, (H, W) in enumerate(SIZES)]
    d_reg = [nc.dram_tensor(f"reg{l}", [IMGS, 4, H, W], f32, kind="ExternalInput")
             for l, (H, W) in enumerate(SIZES)]
    PT2 = 2 * PTC
    d_bm = nc.dram_tensor("boxmeta", [IMGS * M, 5], f32, kind="ExternalInput")
    d_consts = nc.dram_tensor("consts", [128, 4 * TBC + 2 * PT2 + 1], f32,
                              kind="ExternalInput")
    d_out = nc.dram_tensor("out", [4], f32, kind="ExternalOutput")

    with nc.allow_low_precision("fcos bf16 loss arithmetic"), \
         TileContext(nc) as tc:
        import contextlib
        ctx = contextlib.ExitStack()
        with ctx:
            sb = ctx.enter_context(tc.tile_pool(name="sb", bufs=1))
            ps = ctx.enter_context(tc.tile_pool(name="ps", bufs=1, space="PSUM"))

            def T(name, shape):
                return sb.tile(shape, f32, name=name, tag=name)

            # ---- constants to SBUF (single packed DMA)
            NCST = 4 * TBC + 2 * PT2 + 1
            CONSTS = T("CONSTS", [128, NCST])
            nc.sync.dma_start(CONSTS[:], d_consts[:])
            GRIDX = CONSTS[:, 0:TBC]
            GRIDY = CONSTS[:, TBC:2 * TBC]
            HCT = CONSTS[:, 2 * TBC:3 * TBC]
            LCT = CONSTS[:, 3 * TBC:4 * TBC]
            XS = CONSTS[:, 4 * TBC:4 * TBC + PT2]
            YS = CONSTS[:, 4 * TBC + PT2:4 * TBC + 2 * PT2]
            ONES = CONSTS[:, NCST - 1:NCST]

            # ---- box data: [128 = img*64+box, 5] = x0,y0,x2,y2,w
            BM = T("BM", [128, 5])
            nc.scalar.dma_start(BM[:], d_bm[:])
            BX = BM[:, 0:4]
            WS = BM[:, 4:5]
            # payload weights w*coord  [128, 5] cols: D,x0,y0,x2,y2
            W5 = T("W5", [128, 5])
            nc.vector.tensor_copy(W5[:, 0:1], WS)
            for c in range(4):
                nc.vector.tensor_tensor(W5[:, c + 1:c + 2], BX[:, c:c + 1], WS,
                                        Alu.mult)

            # ---- interval tables [128 = img*64+box, 280]
            def axis_tables(GRID, c0, c2, need_v1):
                l = T(f"l{need_v1}", [128, TBC])
                nc.vector.tensor_scalar(l[:], GRID, c0, None, Alu.subtract)
                r = T(f"r{need_v1}", [128, TBC])
                nc.vector.tensor_scalar(r[:], GRID, c2, -1.0,
                                        Alu.subtract, Alu.mult)
                a = T(f"a{need_v1}", [128, TBC])
                nc.vector.tensor_scalar(a[:], l[:], 0.0, None, Alu.is_gt)
                b = T(f"b{need_v1}", [128, TBC])
                nc.vector.tensor_tensor(b[:], l[:], HCT, Alu.is_lt)
                u1 = T(f"u1{need_v1}", [128, TBC])
                nc.vector.tensor_tensor(u1[:], a[:], b[:], Alu.logical_and)
                nc.vector.tensor_scalar(a[:], r[:], 0.0, None, Alu.is_gt)
                nc.vector.tensor_tensor(b[:], r[:], HCT, Alu.is_lt)
                nc.vector.tensor_tensor(a[:], a[:], b[:], Alu.logical_and)
                nc.vector.tensor_tensor(u1[:], u1[:], a[:], Alu.logical_and)
                # u2 = (l<=low)&(r<=low);  u1u2; u1 & ~u2
                nc.vector.tensor_tensor(a[:], l[:], LCT, Alu.is_le)
                nc.vector.tensor_tensor(b[:], r[:], LCT[:], Alu.is_le)
                nc.vector.tensor_tensor(a[:], a[:], b[:], Alu.logical_and)
                u12 = T(f"u12{need_v1}", [128, TBC])
                nc.vector.tensor_tensor(u12[:], u1[:], a[:], Alu.logical_and)
                ua = T(f"ua{need_v1}", [128, TBC])
                nc.vector.tensor_tensor(ua[:], u1[:], u12[:], Alu.subtract)
                return (ua, u12, u1)

            UA, UB, _ = axis_tables(GRIDX, BX[:, 0:1], BX[:, 2:3], "x")
            VA, _, V1 = axis_tables(GRIDY, BX[:, 1:2], BX[:, 3:4], "y")

            # ---- weighted, term-stacked lhsT per (channel, img):
            # rows 0:64 = V1*w (pairs with UA), rows 64:128 = VA*w (pairs
            # with UB) -> one K=128 matmul per tile instead of two K=64.
            LST = [[T(f"lst{c}_{i}", [128, TBC]) for i in range(IMGS)]
                   for c in range(5)]
            RST = [T(f"rst{i}", [128, TBC]) for i in range(IMGS)]
            for i in range(IMGS):
                kb = i * 64
                nc.vector.tensor_copy(RST[i][0:64, :], UA[kb:kb + 64, :])
                nc.vector.tensor_copy(RST[i][64:128, :], UB[kb:kb + 64, :])
                for c in range(5):
                    nc.vector.tensor_scalar(LST[c][i][0:64, :], V1[kb:kb + 64, :],
                                            W5[kb:kb + 64, c:c + 1], None, Alu.mult)
                    nc.vector.tensor_scalar(LST[c][i][64:128, :], VA[kb:kb + 64, :],
                                            W5[kb:kb + 64, c:c + 1], None, Alu.mult)

            # ---- matmul channels into PSUM [128, 512] x5
            # tiles: (name, lhsT y-cols, rhs x-cols, out rows, out cols, M, N)
            # PSUM matmul outputs at base_partition>0 may span at most 32
            # partitions (walrus col_grp rule) -> split L1/L2 into 32-row rows.
            tiles = [
                ("L0a", 0, 0, 0, 0, 128, 160),
                ("L0b", 128, 0, 0, 160, 32, 160),
                ("L1a", 160, 160, 32, 160, 32, 80),
                ("L1b", 192, 160, 64, 160, 32, 80),
                ("L1c", 224, 160, 96, 160, 16, 80),
                ("L2a", 240, 240, 32, 240, 32, 40),
                ("L2b", 272, 240, 64, 240, 8, 40),
            ]
            PCH = [ps.tile([128, 512], f32, name=f"pch{c}", tag=f"pch{c}") for c in range(5)]
            for c in range(5):
                nc.vector.memset(PCH[c][:, 0:PTC], 0.0)

            ACC = T("ACC", [128, 4])
            nc.vector.memset(ACC[:], 0.0)
            pout = ps.tile([4, 1], f32, name="pout", tag="pout")

            pp = ctx.enter_context(tc.tile_pool(name="pp", bufs=2))

            for _ in range(rep):
                # fused two-image per-pixel tensors [128, 640]; rotating pool
                # so consecutive iterations (timing reps) can pipeline
                PCL = pp.tile([128, PT2], f32, name="PCL", tag="PCL")
                PRG = pp.tile([128, 4 * PT2], f32, name="PRG", tag="PRG")
                S = {k: pp.tile([128, PT2], f32, name=f"s{k}", tag=f"s{k}")
                     for k in (0, 1, 2, 3, 4, 8, 9)}
                MB = [pp.tile([128, PT2], f32, name=f"mb{k}", tag=f"mb{k}")
                      for k in range(4)]
                POS = pp.tile([128, PT2], f32, name="POS", tag="POS")
                Q = pp.tile([128, PT2], f32, name="Q", tag="Q")
                IOU = pp.tile([128, PT2], f32, name="IOU", tag="IOU")
                AU = pp.tile([128, PT2], f32, name="AU", tag="AU")
                AI = pp.tile([128, PT2], bf16, name="AI", tag="AI")
                # bf16 mirrors for the 2x DVE mode on the bulk arithmetic
                PRGH = pp.tile([128, 4 * PT2], bf16, name="PRGH", tag="PRGH")
                H = {k: pp.tile([128, PT2], bf16, name=f"h{k}", tag=f"h{k}")
                     for k in range(2, 10)}
                QH = pp.tile([128, PT2], bf16, name="QH", tag="QH")
                NPT = pp.tile([128, 4], f32, name="NPT", tag="NPT")
                nc.vector.memset(PCL[:], -60.0)
                for c in range(4):
                    nc.vector.memset(PRG[:, c * PT2:(c + 1) * PT2], 1.0)
                PRG_r = PRG.rearrange("p (pl c) -> p pl c", pl=4)
                for img in range(IMGS):
                    kb = img * 64   # K row base for this image's tables
                    ib = img * PTC  # col base in fused layout

                    # ---- DMA pixel data for this image
                    r0 = d_reg[0].ap()[img].rearrange("pl h w -> h pl w")
                    r1 = d_reg[1].ap()[img].rearrange("pl h w -> h pl w")
                    r2 = d_reg[2].ap()[img].rearrange("pl h w -> h pl w")
                    nc.sync.dma_start(PRG_r[:, :, ib + 0:ib + 160], r0[0:128])
                    nc.sync.dma_start(PRG_r[0:32, :, ib + 160:ib + 320], r0[128:160])
                    nc.sync.dma_start(PRG_r[32:112, :, ib + 160:ib + 240], r1)
                    nc.sync.dma_start(PRG_r[32:72, :, ib + 240:ib + 280], r2)
                    c0 = d_cls[0].ap()[img, 0]
                    nc.gpsimd.dma_start(PCL[0:128, ib:ib + 160], c0[0:128])
                    nc.gpsimd.dma_start(PCL[0:32, ib + 160:ib + 320], c0[128:160])
                    nc.gpsimd.dma_start(PCL[32:112, ib + 160:ib + 240],
                                        d_cls[1].ap()[img, 0])
                    nc.gpsimd.dma_start(PCL[32:72, ib + 240:ib + 280],
                                        d_cls[2].ap()[img, 0])

                    # ---- matmuls: 5 channels x 7 tiles x 2 terms, then drain
                    for c in range(5):
                        for (tname, yc, xc, orow, ocol, tm, tn) in tiles:
                            out_ap = PCH[c][orow:orow + tm, ocol:ocol + tn]
                            nc.tensor.matmul(
                                out_ap,
                                LST[c][img][:, yc:yc + tm],
                                RST[img][:, xc:xc + tn],
                                start=True, stop=True,
                                tile_position=(0, orow))

                    # per-image decode straight from PSUM into the fused
                    # [128,640] layout; frees PSUM for the next image's MMs
                    D = PCH[0][:, 0:PTC]
                    sl = slice(ib, ib + PTC)
                    nc.vector.tensor_scalar(POS[:, sl], D, float(2.0 ** -112),
                                            0.0, Alu.is_gt, Alu.add,
                                            accum_out=NPT[:, img:img + 1])
                    nc.vector.tensor_scalar(S[0][:, sl], D, float(2.0 ** -115),
                                            None, Alu.max)
                    nc.vector.reciprocal_approx_fast(S[1][:, sl], S[0][:, sl])
                    for k in range(4):
                        nc.vector.tensor_tensor(MB[k][:, sl],
                                                PCH[k + 1][:, 0:PTC],
                                                S[1][:, sl], Alu.mult)
                # ltrb in bf16 (output cast; inputs stay f32)
                TL, TT_, TR, TB = H[2], H[3], H[4], H[5]
                nc.vector.tensor_tensor(TL[:], XS, MB[0][:], Alu.subtract)
                nc.vector.tensor_tensor(TT_[:], YS, MB[1][:], Alu.subtract)
                nc.vector.tensor_tensor(TR[:], MB[2][:], XS, Alu.subtract)
                nc.vector.tensor_tensor(TB[:], MB[3][:], YS, Alu.subtract)
                nc.vector.tensor_copy(PRGH[:], PRG[:])

                PL = PRGH[:, 0:PT2]
                PT = PRGH[:, PT2:2 * PT2]
                PR = PRGH[:, 2 * PT2:3 * PT2]
                PB = PRGH[:, 3 * PT2:4 * PT2]

                # iou pieces (bf16, DVE 2x)
                w_i, h_i = H[6], H[7]
                nc.vector.tensor_tensor(w_i[:], PL, TL[:], Alu.min)
                nc.vector.tensor_tensor(H[8][:], PR, TR[:], Alu.min)
                nc.vector.tensor_tensor(w_i[:], w_i[:], H[8][:], Alu.add)
                nc.vector.tensor_tensor(h_i[:], PT, TT_[:], Alu.min)
                nc.vector.tensor_tensor(H[8][:], PB, TB[:], Alu.min)
                nc.vector.tensor_tensor(h_i[:], h_i[:], H[8][:], Alu.add)
                nc.vector.tensor_scalar(w_i[:], w_i[:], 0.0, None, Alu.max)
                nc.vector.tensor_scalar(h_i[:], h_i[:], 0.0, None, Alu.max)
                nc.vector.tensor_tensor(AI[:], w_i[:], h_i[:], Alu.mult)
                # a_p, a_t, a_u (a_u assembled into f32 for the reciprocal)
                ap_t, at_t = H[6], H[7]
                nc.vector.tensor_tensor(H[8][:], PL, PR, Alu.add)
                nc.vector.tensor_tensor(H[9][:], PT, PB, Alu.add)
                nc.vector.tensor_tensor(ap_t[:], H[8][:], H[9][:], Alu.mult)
                nc.vector.tensor_tensor(H[8][:], TL[:], TR[:], Alu.add)
                nc.vector.tensor_tensor(H[9][:], TT_[:], TB[:], Alu.add)
                nc.vector.tensor_tensor(at_t[:], H[8][:], H[9][:], Alu.mult)
                nc.vector.tensor_tensor(H[8][:], ap_t[:], at_t[:], Alu.add)
                nc.vector.tensor_tensor(AU[:], H[8][:], AI[:], Alu.subtract)
                nc.vector.reciprocal_approx_fast(S[8][:], AU[:])
                nc.vector.tensor_tensor(IOU[:], AI[:], S[8][:], Alu.mult)
                # q = clip(iou,0,1)*pos
                nc.vector.tensor_scalar(Q[:], IOU[:], 1.0, None, Alu.min)
                nc.vector.tensor_tensor(Q[:], Q[:], POS[:], Alu.mult)

                # giou extra: a_e  (w_e,h_e >= 2 always: no clamps needed)
                w_e, h_e = H[6], H[7]
                nc.vector.tensor_tensor(w_e[:], PL, TL[:], Alu.max)
                nc.vector.tensor_tensor(H[8][:], PR, TR[:], Alu.max)
                nc.vector.tensor_tensor(w_e[:], w_e[:], H[8][:], Alu.add)
                nc.vector.tensor_tensor(h_e[:], PT, TT_[:], Alu.max)
                nc.vector.tensor_tensor(H[8][:], PB, TB[:], Alu.max)
                nc.vector.tensor_tensor(h_e[:], h_e[:], H[8][:], Alu.add)
                nc.vector.tensor_tensor(S[8][:], w_e[:], h_e[:], Alu.mult)
                nc.vector.reciprocal_approx_fast(S[9][:], S[8][:])
                nc.vector.tensor_tensor(S[9][:], AU[:], S[9][:], Alu.mult)
                # gl = 2 - iou - a_u/a_e
                nc.vector.tensor_tensor(S[9][:], IOU[:], S[9][:], Alu.add)
                nc.vector.tensor_scalar(S[9][:], S[9][:], -1.0, 2.0,
                                        Alu.mult, Alu.add)
                nc.vector.tensor_tensor(S[9][:], S[9][:], POS[:], Alu.mult)
                nc.scalar.activation(S[8][:], S[9][:], Act.Copy,
                                     accum_out=NPT[:, 2:3])
                nc.vector.tensor_tensor(ACC[:, 1:2], ACC[:, 1:2], NPT[:, 2:3],
                                        Alu.add)
                # npos (from the per-image accum_outs)
                nc.vector.tensor_tensor(NPT[:, 0:1], NPT[:, 0:1], NPT[:, 1:2],
                                        Alu.add)
                nc.vector.tensor_tensor(ACC[:, 2:3], ACC[:, 2:3], NPT[:, 0:1],
                                        Alu.add)

                # ---- varifocal
                # softplus(x) = -Ln(sigmoid(-x)):  s_pos = -L2, s_neg = -L1
                P_, PN, P2A = S[2], S[3], S[4]
                nc.scalar.activation(P_[:], PCL[:], Act.Sigmoid)
                nc.scalar.activation(PN[:], PCL[:], Act.Sigmoid, scale=-1.0)
                nc.scalar.activation(P2A[:], P_[:], Act.Square,
                                     scale=float(np.sqrt(0.75)))
                L1 = H[2]
                nc.scalar.activation(L1[:], P_[:], Act.Ln)
                L2 = H[3]
                nc.scalar.activation(L2[:], PN[:], Act.Ln)
                NEG = H[4]
                nc.vector.tensor_copy(H[5][:], P2A[:])
                nc.vector.tensor_tensor(NEG[:], H[5][:], L2[:], Alu.mult)
                nc.vector.tensor_copy(QH[:], Q[:])
                Q2, QQ = H[6], H[7]
                nc.vector.tensor_tensor(Q2[:], QH[:], QH[:], Alu.mult)
                nc.vector.tensor_tensor(QQ[:], QH[:], Q2[:], Alu.subtract)
                nc.vector.tensor_tensor(Q2[:], Q2[:], L1[:], Alu.mult)
                nc.vector.tensor_tensor(QQ[:], QQ[:], L2[:], Alu.mult)
                nc.vector.tensor_tensor(Q2[:], Q2[:], QQ[:], Alu.add)
                nc.vector.tensor_scalar(H[8][:], QH[:], 0.0, None, Alu.is_gt)
                nc.vector.tensor_tensor(Q2[:], Q2[:], NEG[:], Alu.subtract)
                nc.vector.tensor_tensor(Q2[:], Q2[:], H[8][:], Alu.mult)
                nc.vector.tensor_tensor(Q2[:], Q2[:], NEG[:], Alu.add)
                nc.scalar.activation(S[9][:], Q2[:], Act.Copy, scale=-1.0,
                                     accum_out=NPT[:, 3:4])
                nc.vector.tensor_tensor(ACC[:, 0:1], ACC[:, 0:1], NPT[:, 3:4],
                                        Alu.add)

            # ---- cross-partition reduce: out[c] = sum_p ACC[p, c]
            nc.tensor.matmul(pout[:], ACC[:, 0:4], ONES, start=True, stop=True)
            OUTS = T("OUTS", [4, 1])
            nc.vector.tensor_copy(OUTS[:], pout[:])
            nc.sync.dma_start(d_out.ap().rearrange("(c o) -> c o", o=1), OUTS[:])

    nc.finalize()
    return nc


def _get_nc():
    if "nc" not in _CACHE:
        _CACHE["nc"] = _build_nc()
    return _CACHE["nc"]


def _make_in_maps(cls0, cls1, cls2, reg0, reg1, reg2, gt_boxes):
    GRIDX, GRIDY, HCT, LCT, XS, YS, ONES = _const_tensors()
    consts = np.concatenate([GRIDX, GRIDY, HCT, LCT, XS, YS, ONES], axis=1)
    consts = np.ascontiguousarray(consts, dtype=np.float32)
    in_maps = []
    for core in range(NCORES):
        sl = slice(core * IMGS, (core + 1) * IMGS)
        ws = np.stack([_slot_weights(gt_boxes[i])[0]
                       for i in range(core * IMGS, (core + 1) * IMGS)])
        bm = np.concatenate([gt_boxes[sl].reshape(IMGS * M, 4),
                             ws.reshape(IMGS * M, 1)], axis=1)
        in_maps.append({
            "cls0": np.ascontiguousarray(cls0[sl]),
            "cls1": np.ascontiguousarray(cls1[sl]),
            "cls2": np.ascontiguousarray(cls2[sl]),
            "reg0": np.ascontiguousarray(reg0[sl]),
            "reg1": np.ascontiguousarray(reg1[sl]),
            "reg2": np.ascontiguousarray(reg2[sl]),
            "boxmeta": np.ascontiguousarray(bm, dtype=np.float32),
            "consts": consts,
        })
    return in_maps


def kernel(cls0, cls1, cls2, reg0, reg1, reg2, gt_boxes):
    from concourse.bass_utils import run_bass_kernel_spmd
    nc = _get_nc()
    in_maps = _make_in_maps(np.asarray(cls0, np.float32),
                            np.asarray(cls1, np.float32),
                            np.asarray(cls2, np.float32),
                            np.asarray(reg0, np.float32),
                            np.asarray(reg1, np.float32),
                            np.asarray(reg2, np.float32),
                            np.asarray(gt_boxes, np.float32))
    res = run_bass_kernel_spmd(nc, in_maps, core_ids=list(range(NCORES)))
    acc = np.zeros(3, dtype=np.float64)
    for core in range(NCORES):
        acc += res.results[core]["out"][:3].astype(np.float64)
    cls_sum, reg_sum, npos = acc
    navg = max(1.0, npos / B)
    return (np.float32((cls_sum + reg_sum) / navg),
            np.float32(cls_sum / navg),
            np.float32(reg_sum / navg))

